# revision 6
# baseline (speedup 1.0000x reference)
"""Trainium2 Bass kernel for BasicSelfAttention (B=2, N=2048, C=1024, H=16, Dh=64).

Sharding: 8 cores = 2 batches x 4 head-groups. Core c handles batch c//4 and
heads [4*(c%4), 4*(c%4)+4).

v3 design (vs. v2's all-bf16):
  - qkv projection in fp8-e4m3 DoubleRow (0.5 cyc/row) with a 3-term
    residual split: x ~ xh+xl, W ~ wh+wl (same-scale splits so all terms
    accumulate raw in one PSUM group); qkv = xh@wh + xl@wh + xh@wl.
    Accuracy is bf16-level (the dropped xl@wl term is ~2^-8, incoherent).
  - q/k weight rows are centered host-side (head-mean removed), so the
    device LN needs no mean path at all: var = mean(q_c^2), one multiply.
    W scaled x32 so the fp8 splits stay in normal range; LN self-corrects
    the scale (eps' = eps*32^2), the v-path scale folds into W_proj/32.
  - rstd = exp(-0.5*ln(var/64+eps')) on ACT, batched per nb-pair [128,16].
  - PSUM (8 banks): qp bufs=1 (qkv + proj), sp bufs=2 (S tiles; also
    prologue qkv and tail PV), ov bufs=2 (PV accumulators).
  - deficit-paying filler schedule: each S unit's ACT-minus-PE deficit is
    paid with qkv-term / PV-group / proj-half fillers so the PE never
    starves while ACT grinds exp.
  - dummy matmuls at t=0 ramp the PE p-state during the input DMA wait.
"""

import numpy as np
from collections import deque
from contextlib import ExitStack

import ml_dtypes
import concourse.bass as bass
import concourse.mybir as mybir
import concourse.tile as tile
from concourse import bacc
from concourse.bass_utils import run_bass_kernel_spmd

B, N, C, H, Dh = 2, 2048, 1024, 16, 64
HPC = 4                      # heads per core
NCORES = 8
SCALE = 8.0 / Dh             # 0.125 (use_mup)
EPS = 1e-5
WS = 32.0                    # host weight scale for fp8
EPS2 = EPS * WS * WS

F32 = mybir.dt.float32
BF16 = mybir.dt.bfloat16
E4 = mybir.dt.float8e4
AF = mybir.ActivationFunctionType
OP = mybir.AluOpType
DR = mybir.MatmulPerfMode.DoubleRow
BF = ml_dtypes.bfloat16
E4NP = ml_dtypes.float8_e4m3

NB = N // 128                # 16 row blocks of 128
IB = N // 512                # 4 query super-blocks of 512
WQ = 768                     # 256 q | 256 k | 256 v

_BUILD_CACHE = {}
LAST_RESULT = None


def _bc3(ap2d, inner):
    """[p, g] AP -> [p, g, inner] with stride-0 inner dim."""
    return bass.AP(tensor=ap2d.tensor, offset=ap2d.offset,
                   ap=list(ap2d.ap) + [[0, inner]])


def _build(causal: bool, fast_gb: bool, exp_bias: float):
    nc = bacc.Bacc("TRN2", target_bir_lowering=False, debug=False,
                   num_devices=NCORES)

    xh_e = nc.dram_tensor("xh8", [512, 2 * N], E4, kind="ExternalInput")
    xl_e = nc.dram_tensor("xl8", [512, 2 * N], E4, kind="ExternalInput")
    wh_e = nc.dram_tensor("wh8", [512, 2 * WQ], E4, kind="ExternalInput")
    wl_e = nc.dram_tensor("wl8", [512, 2 * WQ], E4, kind="ExternalInput")
    wp_e = nc.dram_tensor("wp_t", [HPC * Dh, C], BF16, kind="ExternalInput")
    if not fast_gb:
        gt_e = nc.dram_tensor("g_bcast", [128, 512], F32, kind="ExternalInput")
        bt_e = nc.dram_tensor("b_bcast", [128, 512], F32, kind="ExternalInput")
    out_e = nc.dram_tensor("out_p", [N, C], BF16, kind="ExternalOutput")

    with tile.TileContext(nc) as tc, ExitStack() as ctx:
        persist = ctx.enter_context(tc.tile_pool(name="persist", bufs=1))
        ones_t = persist.tile([128, 1], BF16, tag="ones")
        nc.vector.memset(ones_t[:], 1.0)
        eps_t = persist.tile([128, 1], F32, tag="eps")
        nc.vector.memset(eps_t[:], EPS2)
        eb_t = persist.tile([128, 1], F32, tag="ebias")
        nc.vector.memset(eb_t[:], exp_bias)
        dummy_t = persist.tile([128, 512], BF16, tag="dummy")
        nc.gpsimd.memset(dummy_t[:], 0.125)

        # the one ACT table holding Exp+Ln+Copy (id 6), preloaded once
        nc.scalar.add_instruction(mybir.InstLoadActFuncSet(
            name=nc.get_next_instruction_name(), act_func_set_id=6,
            engine=mybir.EngineType.Activation, ins=[], outs=[]))

        # transposed q|k, segments: 0,1 = q head-pairs; 2,3 = k head-pairs
        qkT = persist.tile([128, 4, N], BF16, tag="qkT")
        # transposed normalized attention output, head-pairs, input to proj
        oT = persist.tile([128, 2, N], BF16, tag="oT")

        xh = [persist.tile([128, 2, N], E4, tag=f"xh{p}", name=f"xh{p}")
              for p in range(4)]
        xl = [persist.tile([128, 2, N], E4, tag=f"xl{p}", name=f"xl{p}")
              for p in range(4)]
        wh = [persist.tile([128, 2, WQ], E4, tag=f"wh{p}", name=f"wh{p}")
              for p in range(4)]
        wl = [persist.tile([128, 2, WQ], E4, tag=f"wl{p}", name=f"wl{p}")
              for p in range(4)]
        wp_t = [persist.tile([128, C], BF16, tag=f"wp{p}", name=f"wp{p}")
                for p in range(2)]

        if not fast_gb:
            gt = persist.tile([128, 512], F32, tag="gt")
            bt = persist.tile([128, 512], F32, tag="bt")

        va_pool = ctx.enter_context(tc.tile_pool(name="va", bufs=NB))
        va = [None] * NB

        ptp = ctx.enter_context(tc.tile_pool(name="pt", bufs=40))
        natp = ctx.enter_context(tc.tile_pool(name="nat", bufs=4))
        sqp = ctx.enter_context(tc.tile_pool(name="sq", bufs=4))
        stp = ctx.enter_context(tc.tile_pool(name="st", bufs=6))
        rdp = ctx.enter_context(tc.tile_pool(name="rd", bufs=6))
        osp = ctx.enter_context(tc.tile_pool(name="os", bufs=6))
        obp = ctx.enter_context(tc.tile_pool(name="ob", bufs=6))
        # PSUM (8 banks): qp 1x768f32 (2) + sp 2x1024f32 (4) + ov 2x512f32 (2)
        qp = ctx.enter_context(tc.tile_pool(name="qp", bufs=1, space="PSUM"))
        sp = ctx.enter_context(tc.tile_pool(name="sp", bufs=2, space="PSUM"))
        ov = ctx.enter_context(tc.tile_pool(name="ov", bufs=2, space="PSUM"))

        # ---- PE warmup: dummies ramp the p-state during the DMA wait ----
        dps = ov.tile([128, 512], F32, tag="ov", name="warm")
        for _ in range(7):
            nc.tensor.matmul(dps[:], dummy_t[:, 0:128], dummy_t[:],
                             start=True, stop=True)

        # ---- input DMAs, ordered for startup latency ----
        def drs(e):
            return e[:].rearrange("r (s n) -> r s n", s=2)

        for p in range(4):
            nc.sync.dma_start(wh[p][:], drs(wh_e)[128 * p:128 * (p + 1)])
            nc.sync.dma_start(xh[p][:, :, 0:512],
                              drs(xh_e)[128 * p:128 * (p + 1), :, 0:512])
        for p in range(4):
            nc.sync.dma_start(wl[p][:], drs(wl_e)[128 * p:128 * (p + 1)])
            nc.sync.dma_start(xl[p][:, :, 0:512],
                              drs(xl_e)[128 * p:128 * (p + 1), :, 0:512])
        for p in range(4):
            nc.sync.dma_start(xh[p][:, :, 512:N],
                              drs(xh_e)[128 * p:128 * (p + 1), :, 512:N])
        for p in range(4):
            nc.sync.dma_start(xl[p][:, :, 512:N],
                              drs(xl_e)[128 * p:128 * (p + 1), :, 512:N])
        for p in range(2):
            nc.sync.dma_start(wp_t[p][:], wp_e[128 * p:128 * (p + 1), :])
        if not fast_gb:
            nc.sync.dma_start(gt[:], gt_e[:])
            nc.sync.dma_start(bt[:], bt_e[:])

        # ---- qkv row-block emission, 3 fp8 DoubleRow term passes ----
        qkv_ps = {}              # nb -> live psum tile
        qkv_nat = {}             # nb -> nat tile
        rstd2 = {}               # nb-pair -> stats tile [128,16]

        def emit_qkv_term(nb, term, psum_pool):
            n0 = 128 * nb
            if term == 0:
                qkv_ps[nb] = psum_pool.tile(
                    [128, 1024] if psum_pool is sp else [128, WQ],
                    F32, tag=("sps" if psum_pool is sp else "qkv"),
                    name=f"qkv{nb}")
            qps = qkv_ps[nb]
            xa, wa = ((xh, wh), (xl, wh), (xh, wl))[term]
            for p in range(4):
                st = (term == 0 and p == 0)
                spf = (term == 2 and p == 3)
                nc.tensor.matmul(qps[:, 0:512], xa[p][:, :, n0:n0 + 128],
                                 wa[p][:, :, 0:512], start=st, stop=spf,
                                 perf_mode=DR)
                nc.tensor.matmul(qps[:, 512:WQ], xa[p][:, :, n0:n0 + 128],
                                 wa[p][:, :, 512:WQ], start=st, stop=spf,
                                 perf_mode=DR)

        def emit_qk3(nb):
            n0 = 128 * nb
            nat = qkv_nat.pop(nb)
            pair, off = nb // 2, 8 * (nb % 2)
            qk3 = nat[:, 0:512].rearrange("p (g d) -> p g d", g=8)
            nc.vector.tensor_tensor(qk3, qk3,
                                    _bc3(rstd2[pair][:, off:off + 8], Dh),
                                    op=OP.mult)
            if not fast_gb:
                nc.vector.tensor_tensor(nat[:, 0:512], nat[:, 0:512], gt[:],
                                        op=OP.mult)
                nc.vector.tensor_tensor(nat[:, 0:512], nat[:, 0:512], bt[:],
                                        op=OP.add)
            nc.sync.dma_start(qkT[:, :, n0:n0 + 128], nat[:, 0:512],
                              transpose=True)
            vat = va_pool.tile([128, HPC, Dh + 1], BF16, tag="vat",
                               name=f"vat{nb}")
            nc.gpsimd.tensor_copy(
                vat[:, :, 0:Dh],
                nat[:, 512:768].rearrange("p (h d) -> p h d", h=HPC))
            nc.gpsimd.tensor_copy(vat[:, :, Dh:Dh + 1],
                                  ones_t[:].to_broadcast([128, HPC, 1]))
            va[nb] = vat

        def emit_qkv_post(nb, nat_on_act):
            qps = qkv_ps.pop(nb)
            nat = natp.tile([128, WQ], BF16, tag="nat", name=f"nat{nb}")
            qkv_nat[nb] = nat
            if nat_on_act:
                nc.scalar.activation(nat[:], qps[:, 0:WQ], func=AF.Copy)
            else:
                nc.vector.tensor_copy(nat[:], qps[:, 0:WQ])
            sq = sqp.tile([128, 512], BF16, tag="sq", name=f"sq{nb}")
            nc.vector.tensor_tensor(sq[:], nat[:, 0:512], nat[:, 0:512],
                                    op=OP.mult)
            pair, off = nb // 2, 8 * (nb % 2)
            if nb % 2 == 0:
                rstd2[pair] = stp.tile([128, 16], F32, tag="rstd",
                                       name=f"rstd{pair}")
            r2 = rstd2[pair]
            nc.vector.tensor_reduce(r2[:, off:off + 8],
                                    sq[:].rearrange("p (g d) -> p g d", g=8),
                                    axis=mybir.AxisListType.X, op=OP.add)
            if nb % 2 == 1:
                # rstd pair: exp(-0.5*ln(sqs/64 + eps')) in two ACT ops
                nc.scalar.activation(r2[:], r2[:], func=AF.Ln,
                                     scale=1.0 / Dh, bias=eps_t[:])
                nc.scalar.activation(r2[:], r2[:], func=AF.Exp, scale=-0.5)
                for b in (nb - 1, nb):
                    emit_qk3(b)

        def width(ib, jb):
            if not causal or jb < 4 * ib:
                return 512
            return 512 - 128 * (jb - 4 * ib)

        # ---- S + exp + mask for one (ib, h, jp) ----
        def emit_sjp(ib, h, jp, pts):
            p, off = h // 2, 64 * (h % 2)
            i0 = 512 * ib
            jbs = (2 * jp, 2 * jp + 1)
            ws = [width(ib, jb) for jb in jbs]
            s_ps = sp.tile([128, 1024], F32, tag="sps",
                           name=f"s{ib}_{h}_{jp}")
            c0s = [512 - ws[0], 512]
            for half, jb in enumerate(jbs):
                w = ws[half]
                nc.tensor.matmul(
                    s_ps[:, c0s[half]:c0s[half] + w],
                    qkT[off:off + Dh, 2 + p, 128 * jb:128 * (jb + 1)],
                    qkT[off:off + Dh, p, i0 + 512 - w:i0 + 512],
                    start=True, stop=True)
            pt = ptp.tile([128, 1024], BF16, tag="pt",
                          name=f"pt{ib}_{h}_{jp}")
            ebias = 0.0 if exp_bias == 0.0 else eb_t[:]
            nc.scalar.activation(pt[:, c0s[0]:512 + ws[1]],
                                 s_ps[:, c0s[0]:512 + ws[1]],
                                 func=AF.Exp, scale=SCALE, bias=ebias)
            for half, jb in enumerate(jbs):
                if causal and jb >= 4 * ib:
                    t = 128 * (jb - 4 * ib)
                    c = (t if half == 0 else 512)
                    nc.gpsimd.affine_select(
                        out=pt[:, c:c + 128], in_=pt[:, c:c + 128],
                        compare_op=OP.is_ge, fill=0.0, base=0,
                        pattern=[[1, 128]], channel_multiplier=-1)
            pts[(h, jp)] = pt

        def pt_col(ib, jb, half, g):
            bp = g - 4 * ib
            if half == 0:
                return 128 * bp
            return 512 + 128 * bp - (512 - width(ib, jb))

        def pv_group(ib, g, h, pts, o_ps):
            jmax = g + 1 if causal else NB
            for jb in range(jmax):
                jp, half = jb // 2, jb % 2
                col = pt_col(ib, jb, half, g)
                nc.tensor.matmul(
                    o_ps[:, h, :], pts[(h, jp)][:, col:col + 128],
                    va[jb][:, h, :],
                    start=(jb == 0), stop=(jb == jmax - 1))

        def pv_finish(g, o_t):
            o_ps = o_t[:, 0:HPC * (Dh + 1)].rearrange("p (h d) -> p h d",
                                                      h=HPC)
            rd = rdp.tile([128, HPC, 1], F32, tag="rd", name=f"rd{g}")
            nc.vector.reciprocal(rd[:], o_ps[:, :, Dh:Dh + 1])
            osb = osp.tile([128, HPC, Dh], BF16, tag="osb", name=f"osb{g}")
            nc.vector.tensor_tensor(osb[:], o_ps[:, :, 0:Dh],
                                    _bc3(rd[:, :, 0], Dh), op=OP.mult)
            n0 = 128 * g
            nc.sync.dma_start(oT[:, :, n0:n0 + 128], osb[:],
                              transpose=True)

        # ---- output projection halves (psum via qp pool) ----
        proj_ob = {}

        def emit_proj_half(nb, j2, psum_pool=None):
            n0 = 128 * nb
            if nb not in proj_ob:
                proj_ob[nb] = obp.tile([128, C], BF16, tag="ob",
                                       name=f"ob{nb}")
            ob = proj_ob[nb]
            pool = psum_pool if psum_pool is not None else qp
            pp_t = pool.tile(
                [128, 1024] if pool is sp else
                ([128, 512] if pool is ov else [128, WQ]),
                F32, tag=("sps" if pool is sp else
                          ("ov" if pool is ov else "qkv")),
                name=f"pp{nb}_{j2}")
            pp = pp_t[:, 0:512]
            nc.tensor.matmul(pp[:], oT[:, 0, n0:n0 + 128],
                             wp_t[0][:, 512 * j2:512 * (j2 + 1)],
                             start=True, stop=False)
            nc.tensor.matmul(pp[:], oT[:, 1, n0:n0 + 128],
                             wp_t[1][:, 512 * j2:512 * (j2 + 1)],
                             start=False, stop=True)
            nc.vector.tensor_copy(ob[:, 512 * j2:512 * (j2 + 1)], pp[:])
            if j2 == 1:
                nc.sync.dma_start(out_e[n0:n0 + 128, :], ob[:])
                proj_ob.pop(nb)

        # ================= emission schedule =================
        # prologue: qkv nb0-3 (psum alternates qp / sp), nat on ACT
        for nb in range(4):
            pool = qp if nb % 2 == 0 else sp
            for term in range(3):
                emit_qkv_term(nb, term, pool)
            emit_qkv_post(nb, nat_on_act=True)

        # two filler queues: `must` (qkv for the next phase, drained
        # linearly over the first ~60% of the phase) and `defer`
        # (PV groups / proj halves, paid against the per-unit ACT deficit;
        # leftovers carry into the next phase)
        must = deque()
        defer = deque()
        bal = [0.0]

        def pay_defer(deficit):
            bal[0] += deficit
            while bal[0] > 0 and defer:
                ns, fn = defer.popleft()
                fn()
                bal[0] -= ns

        def add_qkv_fillers(nb):
            for term in range(3):
                must.append((640.0, lambda nb=nb, t=term:
                             emit_qkv_term(nb, t, qp)))
            must.append((50.0, lambda nb=nb:
                         emit_qkv_post(nb, nat_on_act=(nb < 8))))

        pv_state = {}

        def pv_filler(ib, g, h, pts_src):
            jmax = g + 1 if causal else NB
            pe_ns = jmax * 65 * 0.4167

            def grp():
                if h == 0:
                    pv_state[g] = ov.tile([128, 512], F32, tag="ov",
                                          name=f"o{g}")
                o_ps = pv_state[g][:, 0:HPC * (Dh + 1)].rearrange(
                    "p (h d) -> p h d", h=HPC)
                pv_group(ib, g, h, pts_src, o_ps)
                if h == HPC - 1:
                    pv_finish(g, pv_state.pop(g))
                    for j2 in range(2):
                        defer.append((426.0, lambda nb=g, j2=j2:
                                      emit_proj_half(nb, j2)))
            return (pe_ns, grp)

        # phases
        prev_pts = None
        for k in range(IB):
            pts = {}
            jmaxp = 2 * (k + 1) if causal else NB // 2
            if k < IB - 1:
                for nb in range(4 * (k + 1), 4 * (k + 2)):
                    add_qkv_fillers(nb)
            if k >= 1:
                for g in range(4 * (k - 1), 4 * k):
                    for h in range(HPC):
                        defer.append(pv_filler(k - 1, g, h, prev_pts))
            must_total = sum(ns for ns, _ in must)
            must_spent = [0.0]
            U = HPC * jmaxp
            u = 0
            last = (k == IB - 1)
            for h in range(HPC):
                for jp in range(jmaxp):
                    ws = [width(k, 2 * jp), width(k, 2 * jp + 1)]
                    wtot = ws[0] + ws[1]
                    emit_sjp(k, h, jp, pts)
                    u += 1
                    tgt = must_total * min(1.0, u / (0.6 * U))
                    while must_spent[0] < tgt and must:
                        ns, fn = must.popleft()
                        fn()
                        must_spent[0] += ns
                    # deficit = ACT(0.833w+242) - PE(0.4167w)
                    pay_defer(0.4167 * wtot + 242.0)
                if last:
                    # early PV of this phase's first two chunks (ov psum)
                    for g in (4 * k, 4 * k + 1):
                        defer.append(pv_filler(k, g, h, pts))
            while must:
                _, fn = must.popleft()
                fn()
            prev_pts = pts

        # tail: drain defer, then PV of the last two chunks (sp psum),
        # interleaving leftover proj halves between accumulation groups.
        while defer:
            _, fn = defer.popleft()
            fn()
        gs = [4 * IB - 2, 4 * IB - 1]
        tail_ot = {}
        for g in gs:
            tail_ot[g] = sp.tile([128, 1024], F32, tag="sps", name=f"o{g}")
        for h in range(HPC):
            for g in gs:
                o_ps = tail_ot[g][:, 0:HPC * (Dh + 1)].rearrange(
                    "p (h d) -> p h d", h=HPC)
                pv_group(IB - 1, g, h, prev_pts, o_ps)
                if defer:
                    _, fn = defer.popleft()
                    fn()
        for g in gs:
            pv_finish(g, tail_ot[g])
        while defer:
            _, fn = defer.popleft()
            fn()
        for g in gs:
            emit_proj_half(g, 0, qp)
            emit_proj_half(g, 1, ov)
    return nc


def kernel(x, W_qkv, W_proj, b_proj, ln_g, ln_b, causal, _trace=False):
    global LAST_RESULT
    x = np.asarray(x, dtype=np.float32)
    W_qkv = np.asarray(W_qkv, dtype=np.float32)
    W_proj = np.asarray(W_proj, dtype=np.float32)
    b_proj = np.asarray(b_proj, dtype=np.float32)
    ln_g = np.asarray(ln_g, dtype=np.float32)
    ln_b = np.asarray(ln_b, dtype=np.float32)
    causal = bool(int(np.asarray(causal)))

    fast_gb = bool(np.all(ln_g == 1.0) and np.all(ln_b == 0.0))
    exp_bias = 0.0
    if not fast_gb:
        m = float(SCALE * (8.0 * np.abs(ln_g).max() + 8.0 * np.abs(ln_b).max()) ** 2)
        exp_bias = -max(0.0, m - 8.0)

    key = (causal, fast_gb, exp_bias)
    if key not in _BUILD_CACHE:
        nc = _build(causal, fast_gb, exp_bias)
        nc.finalize()
        _BUILD_CACHE[key] = nc
    nc = _BUILD_CACHE[key]

    def pairpack(a):
        # [1024, M] -> rows (256p + 128s + r) -> [512, 2M] with r-major rows
        M = a.shape[1]
        return np.ascontiguousarray(
            a.reshape(4, 2, 128, M).transpose(0, 2, 1, 3).reshape(512, 2 * M))

    def split8(a):
        hi = a.astype(E4NP)
        lo = (a - hi.astype(np.float32)).astype(E4NP)
        return hi, lo

    # center q,k weight rows per head; scale by WS
    Wc = W_qkv.copy()
    for part in range(2):
        blk = Wc[part * C:(part + 1) * C].reshape(H, Dh, C)
        Wc[part * C:(part + 1) * C] = (
            blk - blk.mean(axis=1, keepdims=True)).reshape(C, C)
    Ws = Wc * WS

    xts = []
    for b in range(B):
        xt = np.ascontiguousarray(x[b].T)            # [C, N]
        hi, lo = split8(xt)
        xts.append((pairpack(hi.view(np.uint8)).view(E4NP),
                    pairpack(lo.view(np.uint8)).view(E4NP)))

    in_maps = []
    for c in range(NCORES):
        b, h0 = c // HPC, Dh * HPC * (c % HPC)   # h0 in channel units
        rq = Ws[h0:h0 + 256]
        rk = Ws[C + h0:C + h0 + 256]
        rv = Ws[2 * C + h0:2 * C + h0 + 256]
        w_all = np.concatenate([rq, rk, rv])          # [768, 1024]
        wT = np.ascontiguousarray(w_all.T)            # [1024, 768]
        whv, wlv = split8(wT)
        im = {
            "xh8": xts[b][0],
            "xl8": xts[b][1],
            "wh8": pairpack(whv.view(np.uint8)).view(E4NP),
            "wl8": pairpack(wlv.view(np.uint8)).view(E4NP),
            "wp_t": np.ascontiguousarray(
                (W_proj[:, h0:h0 + 256] / WS).T).astype(BF),
        }
        if not fast_gb:
            gseg = np.tile(ln_g, 8)
            bseg = np.tile(ln_b, 8)
            im["g_bcast"] = np.broadcast_to(gseg, (128, 512)).copy()
            im["b_bcast"] = np.broadcast_to(bseg, (128, 512)).copy()
        in_maps.append(im)

    res = run_bass_kernel_spmd(nc, in_maps, core_ids=list(range(NCORES)),
                               trace=_trace)
    LAST_RESULT = res

    out = np.empty((B, N, C), dtype=np.float32)
    for b in range(B):
        acc = res.results[4 * b]["out_p"].astype(np.float32)
        for c in range(4 * b + 1, 4 * b + 4):
            acc = acc + res.results[c]["out_p"].astype(np.float32)
        out[b] = acc + b_proj
    return out
